# revision 16
# baseline (speedup 1.0000x reference)
"""RGCN-style GNN message passing on 8 TRN2 NeuronCores.

Sharding: nodes partitioned by dst-ownership (12500/core); each core owns the
incoming edges of its nodes. Per layer, every core computes the relational
transform x3 = h @ W_r for its node shard, shards are AllGathered into a full
bf16 table, and each core gathers 256B rows (indirect DMA, int32 offsets) for
its edges, scales by norm (DVE), and segment-sums into PSUM via identity
matmuls over degree-sorted "level" slices. GRU / BatchNorm / heads run
feature-major on-chip; BN batch stats are AllReduced across cores.
"""

import math
import numpy as np
from ml_dtypes import bfloat16

# ----------------------------------------------------------------- config

def make_cfg(n_nodes=100_000, n_edges=1_600_000, n_cores=8):
    d = 64
    nloc = n_nodes // n_cores
    npad = ((nloc + 127) // 128) * 128
    if (npad // 128) % 2:
        npad += 128
    nt = npad // 128            # node tiles per core
    band = npad // 2            # columns per band (2-band feature-major)
    return dict(
        NCORES=n_cores, N=n_nodes, E=n_edges, D=d, R=3,
        NLOC=nloc, NPAD=npad, NT=nt, BAND=band,
        SH_ROWS=npad * 3,                 # x3 shard rows per core
        TBL_ROWS=n_cores * npad * 3,      # full table rows
        PASS0=min(64, nt),                # seg-cols in psum pass 0
        CHUNK_COLS=64,                    # gather chunk width (grid cols)
        NCHUNK=512,                       # node-col chunk for dense matmuls
        EPS=1e-5,
    )


# ------------------------------------------------------------- host planner

def make_plan(cfg, inputs):
    """All host-side index/schedule prep. Returns plan dict with per-core
    input arrays and the common (SPMD-identical) schedule."""
    C = cfg["NCORES"]; NLOC = cfg["NLOC"]; NPAD = cfg["NPAD"]; R = cfg["R"]
    NT = cfg["NT"]
    src = np.asarray(inputs["src"]).astype(np.int64)
    dst = np.asarray(inputs["dst"]).astype(np.int64)
    rel = np.asarray(inputs["rel_type"]).astype(np.int64)
    norm = np.asarray(inputs["norm"]).astype(np.float32)

    # --- per-core degree + permutation (pi sorts local nodes by deg desc)
    core_of_dst = dst // NLOC
    orders, qofs, degs_q = [], [], []
    for c in range(C):
        m = core_of_dst == c
        u = dst[m] - c * NLOC
        deg = np.bincount(u, minlength=NPAD).astype(np.int64)
        order = np.argsort(-deg, kind="stable")       # q -> local node
        qof = np.empty(NPAD, np.int64)
        qof[order] = np.arange(NPAD)
        orders.append(order); qofs.append(qof); degs_q.append(deg[order])
    qof_all = np.stack(qofs)                          # [C, NPAD]

    # --- common level structure: G_j = #segs with deg > j  (per core -> max)
    maxJ = int(max(dq[0] for dq in degs_q))
    Cj = np.zeros(maxJ, np.int64)
    for dq in degs_q:
        G = np.searchsorted(-dq, -np.arange(1, maxJ + 1), side="right")
        Cj = np.maximum(Cj, (G + 127) // 128)
    Cj = Cj.astype(np.int64)                          # cols per level (common)

    # --- grid columns: (pass, level j, seg-col sc) ordered pass-major
    P0 = cfg["PASS0"]
    gridcols = []
    for pss, (lo, hi) in enumerate([(0, P0), (P0, NT)]):
        if lo >= hi:
            continue
        for j in range(maxJ):
            for sc in range(lo, min(hi, Cj[j])):
                gridcols.append((pss, j, sc))
    CT = len(gridcols)

    # --- per-core idx/norm grids
    gidx_all, normb_all = [], []
    for c in range(C):
        m = np.where(core_of_dst == c)[0]
        u = dst[m] - c * NLOC
        q = qof_all[c][u]
        o = np.argsort(q, kind="stable")
        e_sel = m[o]; q_s = q[o]
        cnt = np.bincount(q_s, minlength=NPAD)
        starts = np.zeros(NPAD + 1, np.int64)
        np.cumsum(cnt, out=starts[1:])
        # table row for each owned edge (sorted by q): src pi-position
        es = src[e_sel]; cs = es // NLOC
        qs = qof_all[cs, es - cs * NLOC]
        trow = (cs * NPAD * R + qs * R + rel[e_sel]).astype(np.int64)
        nv = norm[e_sel]

        gidx = np.zeros((128, CT), np.int32)
        ngrid = np.zeros((128, CT), np.float32)
        dq = degs_q[c]
        p_arange = np.arange(128)
        for k, (pss, j, sc) in enumerate(gridcols):
            qq = sc * 128 + p_arange
            act = dq[qq] > j
            idx = starts[qq] + j
            idx[~act] = 0
            gidx[:, k] = np.where(act, trow[idx], 0)
            ngrid[:, k] = np.where(act, nv[idx], 0.0)
        gidx_all.append(gidx)
        normb = np.repeat(ngrid[:, :, None], cfg["D"], axis=2)
        normb_all.append(normb.reshape(128, CT * cfg["D"]).astype(bfloat16))

    # --- merge-MM schedule per gather chunk (common across cores)
    CH = cfg["CHUNK_COLS"]
    nchunks = (CT + CH - 1) // CH
    # count MMs per psum tile to place start/stop flags
    # psum tile id: (pass, t8) where t8 = sc//8 - (pass base)//8
    touches = {}
    for k, (pss, j, sc) in enumerate(gridcols):
        key = (pss, sc // 8)
        touches[key] = touches.get(key, 0) + 1
    mm_sched = [[] for _ in range(nchunks)]
    seen = {}
    for k, (pss, j, sc) in enumerate(gridcols):
        ch = k // CH
        key = (pss, sc // 8)
        seen[key] = seen.get(key, 0) + 1
        mm_sched[ch].append(dict(
            col=k - ch * CH, pss=pss, t8=sc // 8, pcol=sc % 8,
            start=seen[key] == 1, stop=seen[key] == touches[key],
        ))
    # psum tiles per pass: t8 range and seg-col width
    pass_tiles = []
    for pss, (lo, hi) in enumerate([(0, P0), (P0, NT)]):
        if lo >= hi:
            pass_tiles.append([])
            continue
        tiles = []
        for t8 in range(lo // 8, (hi + 7) // 8):
            w = min(hi, (t8 + 1) * 8) - t8 * 8
            if (pss, t8) in touches:
                tiles.append((t8, w))
        pass_tiles.append(tiles)

    return dict(
        orders=orders, qof_all=qof_all, CT=CT, maxJ=maxJ,
        gridcols=gridcols, mm_sched=mm_sched, pass_tiles=pass_tiles,
        nchunks=nchunks, gidx=gidx_all, normb=normb_all,
    )


def make_in_maps(cfg, plan, inputs):
    C = cfg["NCORES"]; NLOC = cfg["NLOC"]; NPAD = cfg["NPAD"]
    BAND = cfg["BAND"]; D = cfg["D"]
    f32 = np.float32
    v = np.asarray(inputs["v"]).astype(f32)

    def bn_pack(*names):
        return np.stack([np.asarray(inputs[n]).astype(f32) for n in names],
                        axis=1)  # [64, k]

    wih = np.asarray(inputs["gru_Wih"]).astype(f32)   # [192, 64]
    whh = np.asarray(inputs["gru_Whh"]).astype(f32)
    bih = np.asarray(inputs["gru_bih"]).astype(f32)
    bhh = np.asarray(inputs["gru_bhh"]).astype(f32)
    relw = np.asarray(inputs["rel_weight"]).astype(f32)

    ident = np.eye(128, dtype=f32)
    p2 = np.zeros((128, 64), f32)
    p2[np.arange(64), np.arange(64)] = 1.0
    p2[np.arange(64) + 64, np.arange(64)] = 1.0
    p3 = np.zeros((64, 128), f32)
    p3[np.arange(64), np.arange(64)] = 1.0
    p3[np.arange(64), np.arange(64) + 64] = 1.0

    def dup2(w):   # [64, x] -> [128, x] duplicated halves (band-B operands)
        return np.concatenate([w, w], axis=0)

    relw_cat = np.concatenate([relw[r] for r in range(3)], axis=1)  # [64,192]

    common = {
        "embT": dup2(np.asarray(inputs["emb_W"]).astype(f32).T),    # [128,64]
        "emb_b": np.asarray(inputs["emb_b"]).astype(f32)[None, :],
        "relW": dup2(relw_cat).astype(bfloat16),                    # [128,192]
        "WihT_r": dup2(wih[0:64].T).astype(bfloat16),               # [128,64]
        "WihT_z": dup2(wih[64:128].T).astype(bfloat16),
        "WihT_n": dup2(wih[128:192].T).astype(bfloat16),
        "WhhT_r": dup2(whh[0:64].T).astype(bfloat16),
        "WhhT_z": dup2(whh[64:128].T).astype(bfloat16),
        "WhhT_n": dup2(whh[128:192].T).astype(bfloat16),
        "b_r": (bih[0:64] + bhh[0:64])[None, :].astype(bfloat16),
        "b_z": (bih[64:128] + bhh[64:128])[None, :].astype(bfloat16),
        "b_in": bih[128:192][None, :].astype(bfloat16),
        "b_hn": bhh[128:192][None, :].astype(bfloat16),
        "idI": ident.astype(bfloat16),
        "idT": ident,
        "P2": p2, "P3": p3,
        "bn_emb": bn_pack("emb_gamma", "emb_beta"),                 # [64,2]
        "bn_ker": bn_pack("ker_gamma", "ker_beta"),
        "bn_head": np.concatenate([
            bn_pack("a_gamma", "a_beta"), bn_pack("b_gamma", "b_beta")],
            axis=0),                                                # [128,2]
        "catW1T": dup2(np.concatenate([
            np.asarray(inputs["a_W1"]).astype(f32).T,
            np.asarray(inputs["b_W1"]).astype(f32).T], axis=1)
            ).astype(bfloat16),                                     # [128,128]
        "catb1": np.concatenate([
            np.asarray(inputs["a_b1"]).astype(f32),
            np.asarray(inputs["b_b1"]).astype(f32)])[None, :].astype(bfloat16),
        "aW2T": dup2(np.asarray(inputs["a_W2"]).astype(f32).T).astype(bfloat16),
        "bW2T": dup2(np.asarray(inputs["b_W2"]).astype(f32).T).astype(bfloat16),
        "a_b2": np.asarray(inputs["a_b2"]).astype(f32)[None, :],
        "b_b2": np.asarray(inputs["b_b2"]).astype(f32)[None, :],
    }

    in_maps = []
    for c in range(C):
        order = plan["orders"][c]
        vloc = np.zeros((NPAD, D), f32)
        vloc[:NLOC] = v[c * NLOC:(c + 1) * NLOC]
        vperm = vloc[order]                        # pi-ordered rows
        vT = np.zeros((128, BAND), f32)
        vT[0:64, :] = vperm[:BAND].T
        vT[64:128, :] = vperm[BAND:].T
        m = dict(common)
        m["vT"] = vT
        m["gidx"] = plan["gidx"][c]
        m["normb"] = plan["normb"][c]
        in_maps.append(m)
    return in_maps


# --------------------------------------------------------------- builder

def build_program(cfg, plan):
    import concourse.bass as bass
    import concourse.bacc as bacc
    import concourse.tile as tile
    from concourse import mybir
    from contextlib import ExitStack

    dt = mybir.dt
    AF = mybir.ActivationFunctionType
    OP = mybir.AluOpType
    C = cfg["NCORES"]; NPAD = cfg["NPAD"]; NT = cfg["NT"]; BAND = cfg["BAND"]
    D = cfg["D"]; R = cfg["R"]; CT = plan["CT"]; CH = cfg["CHUNK_COLS"]
    NC512 = cfg["NCHUNK"]
    NTB = NT // 2                     # tiles per band
    GROUPS = [[i for i in range(C)]]
    inv_n = 1.0 / cfg["N"]

    nc = bacc.Bacc("TRN2", target_bir_lowering=False, debug=False,
                   num_devices=C)

    def din(name, shape, d=dt.float32):
        return nc.dram_tensor(name, list(shape), d, kind="ExternalInput")

    vT_d = din("vT", [128, BAND])
    gidx_d = din("gidx", [128, CT], dt.int32)
    normb_d = din("normb", [128, CT * D], dt.bfloat16)
    embT_d = din("embT", [128, 64]); embb_d = din("emb_b", [1, 64])
    relW_d = din("relW", [128, R * 64], dt.bfloat16)
    wih = {g: din(f"WihT_{g}", [128, 64], dt.bfloat16) for g in "rzn"}
    whh = {g: din(f"WhhT_{g}", [128, 64], dt.bfloat16) for g in "rzn"}
    b_r = din("b_r", [1, 64], dt.bfloat16); b_z = din("b_z", [1, 64], dt.bfloat16)
    b_in = din("b_in", [1, 64], dt.bfloat16); b_hn = din("b_hn", [1, 64], dt.bfloat16)
    idI_d = din("idI", [128, 128], dt.bfloat16)
    idT_d = din("idT", [128, 128])
    P2_d = din("P2", [128, 64]); P3_d = din("P3", [64, 128])
    bn_emb_d = din("bn_emb", [64, 2]); bn_ker_d = din("bn_ker", [64, 2])
    bn_head_d = din("bn_head", [128, 2])
    catW1T_d = din("catW1T", [128, 128], dt.bfloat16)
    catb1_d = din("catb1", [1, 128], dt.bfloat16)
    aW2T_d = din("aW2T", [128, 2], dt.bfloat16)
    bW2T_d = din("bW2T", [128, 21], dt.bfloat16)
    ab2_d = din("a_b2", [1, 2]); bb2_d = din("b_b2", [1, 21])

    out_d = nc.dram_tensor("out", [64, NPAD], dt.float32, kind="ExternalOutput")

    shard_d = nc.dram_tensor("x3_shard", [cfg["SH_ROWS"], D], dt.bfloat16)
    table_d = nc.dram_tensor("x3_table", [cfg["TBL_ROWS"], D], dt.bfloat16)
    st_in = [nc.dram_tensor(f"st_in{i}", [128, 2], dt.float32) for i in range(3)]
    st_out = [nc.dram_tensor(f"st_out{i}", [128, 2], dt.float32) for i in range(3)]

    with tile.TileContext(nc) as tc, ExitStack() as ctx:
        per = ctx.enter_context(tc.tile_pool(name="per", bufs=1))
        consts = ctx.enter_context(tc.tile_pool(name="consts", bufs=1))

        def load_const(dram, shape, d=dt.float32, tag=None):
            t = consts.tile(shape, d, tag=tag or dram.name)
            nc.sync.dma_start(t[:], dram.ap()[:])
            return t

        idI = load_const(idI_d, [128, 128], dt.bfloat16)
        idT = load_const(idT_d, [128, 128])
        P2 = load_const(P2_d, [128, 64]); P3 = load_const(P3_d, [64, 128])
        embT = load_const(embT_d, [128, 64]); embb = load_const(embb_d, [1, 64])
        relW = load_const(relW_d, [128, R * 64], dt.bfloat16)
        wihs = {g: load_const(wih[g], [128, 64], dt.bfloat16) for g in "rzn"}
        whhs = {g: load_const(whh[g], [128, 64], dt.bfloat16) for g in "rzn"}
        brz = {"r": load_const(b_r, [1, 64], dt.bfloat16),
               "z": load_const(b_z, [1, 64], dt.bfloat16)}
        bin_ = load_const(b_in, [1, 64], dt.bfloat16)
        bhn = load_const(b_hn, [1, 64], dt.bfloat16)
        bn_emb = load_const(bn_emb_d, [64, 2]); bn_ker = load_const(bn_ker_d, [64, 2])
        bn_head = load_const(bn_head_d, [128, 2])
        catW1T = load_const(catW1T_d, [128, 128], dt.bfloat16)
        catb1 = load_const(catb1_d, [1, 128], dt.bfloat16)
        aW2T = load_const(aW2T_d, [128, 2], dt.bfloat16)
        bW2T = load_const(bW2T_d, [128, 21], dt.bfloat16)
        ab2 = load_const(ab2_d, [1, 2]); bb2 = load_const(bb2_d, [1, 21])
        ones_bf = consts.tile([1, NC512], dt.bfloat16, tag="ones_bf")
        nc.vector.memset(ones_bf[:], 1.0)
        ones_f = consts.tile([1, NC512], dt.float32, tag="ones_f")
        nc.vector.memset(ones_f[:], 1.0)

        # persistent activations
        h0T = per.tile([128, BAND], dt.bfloat16, tag="h0T")
        hA = per.tile([128, BAND], dt.bfloat16, tag="hA")
        hB = per.tile([128, BAND], dt.bfloat16, tag="hB")

        nchunk_cols = [min(NC512, BAND - i * NC512)
                       for i in range((BAND + NC512 - 1) // NC512)]

        def for_chunks():
            for b in range(2):
                for ci, w in enumerate(nchunk_cols):
                    yield b, ci * NC512, w

        def bias_mm(psum_ap, lhsT_bias, w, first, ones=None):
            nc.tensor.matmul(psum_ap, lhsT_bias, (ones or ones_bf)[:, :w],
                             start=first, stop=True)

        # BN helper: stats of x_s ([128, F] fold2=True folds p and p+64),
        # returns (scale[128,1], bias[128,1]) sbuf tiles.
        bn_ctr = [0]

        def bn_stats_apply(pool, psum_pool, x_s, F, gamma_beta, fold2, stid,
                           scratch=None):
            s1 = pool.tile([128, 1], dt.float32, tag=f"s1_{stid}")
            s2 = pool.tile([128, 1], dt.float32, tag=f"s2_{stid}")
            scr = scratch if scratch is not None else pool.tile(
                [128, F], dt.float32, tag="bn_scratch")
            nc.scalar.activation(scr[:], x_s[:], AF.Copy, accum_out=s1[:])
            nc.scalar.activation(scr[:], x_s[:], AF.Square, accum_out=s2[:])
            pack = pool.tile([128, 2], dt.float32, tag=f"pk_{stid}")
            nc.vector.tensor_copy(pack[:, 0:1], s1[:])
            nc.vector.tensor_copy(pack[:, 1:2], s2[:])
            if fold2:
                ps = psum_pool.tile([64, 2], dt.float32, tag="bn_ps")
                nc.tensor.matmul(ps[:], P2[:], pack[:], start=True, stop=True)
                red = pool.tile([128, 2], dt.float32, tag=f"red_{stid}")
                nc.vector.memset(red[:], 0.0)
                nc.vector.tensor_copy(red[0:64, :], ps[:])
                nc.sync.dma_start(st_in[stid].ap()[:], red[:])
            else:
                nc.sync.dma_start(st_in[stid].ap()[:], pack[:])
            nc.gpsimd.collective_compute(
                "AllReduce", OP.add, replica_groups=GROUPS,
                ins=[st_in[stid].ap()[:]], outs=[st_out[stid].ap()[:]])
            P = 64 if fold2 else 128
            tot = pool.tile([P, 2], dt.float32, tag=f"tot_{stid}")
            nc.sync.dma_start(tot[:], st_out[stid].ap()[0:P, :])
            mu = pool.tile([P, 1], dt.float32, tag=f"mu_{stid}")
            var = pool.tile([P, 1], dt.float32, tag=f"var_{stid}")
            t0 = pool.tile([P, 1], dt.float32, tag=f"t0_{stid}")
            nc.scalar.mul(mu[:], tot[:, 0:1], inv_n)
            nc.scalar.mul(var[:], tot[:, 1:2], inv_n)
            nc.scalar.square(t0[:], mu[:])
            nc.vector.tensor_sub(var[:], var[:], t0[:])
            nc.vector.tensor_scalar_add(var[:], var[:], float(cfg["EPS"]))
            nc.scalar.sqrt(var[:], var[:])
            nc.vector.reciprocal(var[:], var[:])        # 1/sqrt(var+eps)
            sc = pool.tile([P, 1], dt.float32, tag=f"sc_{stid}")
            bi = pool.tile([P, 1], dt.float32, tag=f"bi_{stid}")
            nc.vector.tensor_mul(sc[:], gamma_beta[0:P, 0:1], var[:])
            nc.vector.tensor_mul(t0[:], mu[:], sc[:])
            nc.vector.tensor_sub(bi[:], gamma_beta[0:P, 1:2], t0[:])
            if fold2:
                ps2 = psum_pool.tile([128, 2], dt.float32, tag="bn_ps2")
                pk2 = pool.tile([64, 2], dt.float32, tag=f"pk2_{stid}")
                nc.vector.tensor_copy(pk2[:, 0:1], sc[:])
                nc.vector.tensor_copy(pk2[:, 1:2], bi[:])
                nc.tensor.matmul(ps2[:], P3[:], pk2[:], start=True, stop=True)
                sc128 = pool.tile([128, 1], dt.float32, tag=f"scD_{stid}")
                bi128 = pool.tile([128, 1], dt.float32, tag=f"biD_{stid}")
                nc.vector.tensor_copy(sc128[:], ps2[:, 0:1])
                nc.vector.tensor_copy(bi128[:], ps2[:, 1:2])
                return sc128, bi128
            return sc, bi

        # ---------------- embedding: h0 = relu(BN(v @ embW.T + emb_b))
        with tc.tile_pool(name="emb", bufs=1) as ep, \
             tc.tile_pool(name="emb_ps", bufs=2, space="PSUM") as epp:
            vS = ep.tile([128, BAND], dt.float32, tag="vS")
            nc.sync.dma_start(vS[:], vT_d.ap()[:])
            xS = ep.tile([128, BAND], dt.float32, tag="xS")
            for b, c0, w in for_chunks():
                ps = epp.tile([64, NC512], dt.float32, tag="emb_ps")
                pv = ps[:, :w] if w != NC512 else ps[:]
                nc.tensor.matmul(pv, embT[64 * b:64 * b + 64, :],
                                 vS[64 * b:64 * b + 64, c0:c0 + w],
                                 start=True, stop=False)
                nc.tensor.matmul(pv, embb[:], ones_f[:, :w],
                                 start=False, stop=True)
                nc.vector.tensor_copy(xS[64 * b:64 * b + 64, c0:c0 + w], pv)
            # zero pad nodes (pi-tail = band B tail cols)
            padn = NPAD - cfg["NLOC"]
            if padn:
                nc.vector.memset(xS[64:128, BAND - padn:BAND], 0.0)
            sc, bi = bn_stats_apply(ep, epp, xS, BAND, bn_emb, True, 0,
                                    scratch=h0T)
            nc.scalar.activation(h0T[:], xS[:], AF.Relu, bias=bi[:], scale=sc[:])

        # ---------------- two GNN layers
        for layer in range(2):
            srcT = h0T if layer == 0 else hA
            hidT = None if layer == 0 else hA
            outT = hA if layer == 0 else hB

            # ---- x3 shard: per node tile, 3 rel matmuls, stage + DMA out
            with tc.tile_pool(name="x3", bufs=1) as xp, \
                 tc.tile_pool(name="x3_ps", bufs=2, space="PSUM") as xpp:
                stage = xp.tile([128, NT * R * 64], dt.bfloat16, tag="x3_stage")
                for t in range(NT):
                    b, tt = (0, t) if t < NTB else (1, t - NTB)
                    lhs = srcT[64 * b:64 * b + 64, tt * 128:(tt + 1) * 128]
                    ps = xpp.tile([128, R * 64], dt.float32, tag="x3ps")
                    for r in range(R):
                        nc.tensor.matmul(
                            ps[:, r * 64:(r + 1) * 64], lhs,
                            relW[64 * b:64 * b + 64, r * 64:(r + 1) * 64],
                            start=True, stop=True)
                    nc.vector.tensor_copy(
                        stage[:, t * R * 64:(t + 1) * R * 64], ps[:])
                nc.sync.dma_start(
                    shard_d.ap().rearrange("(t p r) f -> p t (r f)",
                                           p=128, r=R)[:],
                    stage[:].rearrange("p (t rf) -> p t rf", rf=R * 64)[:])
            nc.gpsimd.collective_compute(
                "AllGather", OP.bypass, replica_groups=GROUPS,
                ins=[shard_d.ap()[:]], outs=[table_d.ap()[:]])

            # ---- gather + scale + psum merge -> swh_s [128, NT*64] f32
            with tc.tile_pool(name="gat", bufs=1) as gp:
              with tc.tile_pool(name="gat2", bufs=2) as gp2:
                swh = gp.tile([128, NT * 64], dt.float32, tag="swh")
                for pss in range(2):
                  tiles = plan["pass_tiles"][pss]
                  if not tiles:
                      continue
                  with tc.tile_pool(name=f"mps_{layer}_{pss}", bufs=1,
                                    space="PSUM") as mpp:
                    ptile = {t8: mpp.tile([128, w * 64], dt.float32,
                                          name=f"mps{pss}_{t8}",
                                          tag=f"mps{pss}_{t8}")
                             for t8, w in tiles}
                    for ch in range(plan["nchunks"]):
                        mms = [m for m in plan["mm_sched"][ch]
                               if m["pss"] == pss]
                        if not mms:
                            continue
                        c0 = ch * CH
                        cw = min(CH, CT - c0)
                        it = gp2.tile([128, CH], dt.int32, tag="idx")
                        nc.sync.dma_start(it[:, :cw], gidx_d.ap()[:, c0:c0 + cw])
                        gb = gp2.tile([128, CH * D], dt.bfloat16, tag="gbuf")
                        nc.gpsimd.indirect_dma_start(
                            gb[:, :cw * D], None, table_d.ap()[:],
                            bass.IndirectOffsetOnAxis(ap=it[:, :cw], axis=0))
                        nb = gp2.tile([128, CH * D], dt.bfloat16, tag="nbuf")
                        nc.sync.dma_start(
                            nb[:, :cw * D],
                            normb_d.ap()[:, c0 * D:(c0 + cw) * D])
                        mb = gp2.tile([128, CH * D], dt.bfloat16, tag="mbuf")
                        nc.vector.tensor_mul(mb[:, :cw * D], gb[:, :cw * D],
                                             nb[:, :cw * D])
                        # group consecutive cols with same (t8) into one MM
                        runs = []
                        for m in mms:
                            if (runs and runs[-1]["t8"] == m["t8"]
                                    and runs[-1]["col0"] + runs[-1]["n"] == m["col"]
                                    and runs[-1]["pcol"] + runs[-1]["n"] == m["pcol"]):
                                runs[-1]["n"] += 1
                                runs[-1]["stop"] |= m["stop"]
                            else:
                                runs.append(dict(t8=m["t8"], col0=m["col"],
                                                 pcol=m["pcol"], n=1,
                                                 start=m["start"],
                                                 stop=m["stop"]))
                        for rn in runs:
                            pt = ptile[rn["t8"]]
                            nc.tensor.matmul(
                                pt[:, rn["pcol"] * 64:(rn["pcol"] + rn["n"]) * 64],
                                idI[:],
                                mb[:, rn["col0"] * 64:(rn["col0"] + rn["n"]) * 64],
                                start=rn["start"], stop=rn["stop"],
                                skip_group_check=True)
                    for t8, w in tiles:
                        nc.vector.tensor_copy(
                            swh[:, t8 * 8 * 64:(t8 * 8 + w) * 64],
                            ptile[t8][:])

                # ---- transpose swh -> swhT (2-band bf16)
                swhT = gp.tile([128, BAND], dt.bfloat16, tag="swhT")
                with tc.tile_pool(name="tr_ps", bufs=4, space="PSUM") as tpp:
                    # transpose-mode matmul must write PSUM at partition 0:
                    # for band B, widen the input window by 64 cols so the
                    # features land on output partitions 64..127.
                    for t in range(NT):
                        b, tt = (0, t) if t < NTB else (1, t - NTB)
                        ps = tpp.tile([128, 128], dt.float32, tag="trps")
                        if b == 0:
                            nc.tensor.transpose(
                                ps[0:64, :], swh[:, t * 64:(t + 1) * 64],
                                idT[:])
                        else:
                            nc.tensor.transpose(
                                ps[:], swh[:, t * 64 - 64:(t + 1) * 64],
                                idT[:])
                        nc.scalar.activation(
                            swhT[64 * b:64 * b + 64, tt * 128:(tt + 1) * 128],
                            ps[64 * b:64 * b + 64, :], AF.Copy)

                # ---- GRU
                with tc.tile_pool(name="gru", bufs=1) as grp, \
                     tc.tile_pool(name="gru_ps", bufs=1, space="PSUM") as gpp:
                    r_s = grp.tile([128, BAND], dt.bfloat16, tag="r_s")
                    z_s = grp.tile([128, BAND], dt.bfloat16, tag="z_s")
                    in_s = grp.tile([128, BAND], dt.bfloat16, tag="in_s")
                    hn_s = grp.tile([128, BAND], dt.bfloat16, tag="hn_s")
                    for b, c0, w in for_chunks():
                        sl = (slice(64 * b, 64 * b + 64), slice(c0, c0 + w))
                        for gate, dest in (("r", r_s), ("z", z_s)):
                            ps = gpp.tile([128, NC512], dt.float32,
                                          tag=f"ps_{gate}")
                            pv = ps[64 * b:64 * b + 64, :w]
                            nc.tensor.matmul(pv, wihs[gate][64 * b:64 * b + 64, :], swhT[sl],
                                             start=True, stop=False)
                            if hidT is not None:
                                nc.tensor.matmul(pv, whhs[gate][64 * b:64 * b + 64, :], hidT[sl],
                                                 start=False, stop=False)
                            bias_mm(pv, brz[gate], w, False)
                            nc.vector.tensor_copy(dest[sl], pv)
                        ps = gpp.tile([128, NC512], dt.float32, tag="ps_in")
                        pv = ps[64 * b:64 * b + 64, :w]
                        nc.tensor.matmul(pv, wihs["n"][64 * b:64 * b + 64, :], swhT[sl],
                                         start=True, stop=False)
                        bias_mm(pv, bin_, w, False)
                        nc.vector.tensor_copy(in_s[sl], pv)
                        ps = gpp.tile([128, NC512], dt.float32, tag="ps_hn")
                        pv = ps[64 * b:64 * b + 64, :w]
                        if hidT is not None:
                            nc.tensor.matmul(pv, whhs["n"][64 * b:64 * b + 64, :], hidT[sl],
                                             start=True, stop=False)
                            bias_mm(pv, bhn, w, False)
                        else:
                            bias_mm(pv, bhn, w, True)
                        nc.vector.tensor_copy(hn_s[sl], pv)
                    nc.scalar.activation(r_s[:], r_s[:], AF.Sigmoid)
                    nc.scalar.activation(z_s[:], z_s[:], AF.Sigmoid)
                    nc.vector.tensor_mul(hn_s[:], r_s[:], hn_s[:])
                    nc.vector.tensor_add(in_s[:], in_s[:], hn_s[:])
                    nc.scalar.activation(in_s[:], in_s[:], AF.Tanh)   # n
                    if hidT is not None:
                        nc.vector.tensor_sub(hn_s[:], hidT[:], in_s[:])
                        nc.vector.tensor_mul(hn_s[:], z_s[:], hn_s[:])
                        nc.vector.tensor_add(outT[:], in_s[:], hn_s[:])
                    else:
                        # h' = (1-z)*n = n - z*n
                        nc.vector.tensor_mul(hn_s[:], z_s[:], in_s[:])
                        nc.vector.tensor_sub(outT[:], in_s[:], hn_s[:])

        # ---------------- hk = BN(h2); heads
        with tc.tile_pool(name="fin", bufs=1) as fp, \
             tc.tile_pool(name="fin_ps", bufs=2, space="PSUM") as fpp:
            padn = NPAD - cfg["NLOC"]
            if padn:
                nc.vector.memset(hB[64:128, BAND - padn:BAND], 0.0)
            hkT = fp.tile([128, BAND], dt.bfloat16, tag="hkT")
            sc, bi = bn_stats_apply(fp, fpp, hB, BAND, bn_ker, True, 1,
                                    scratch=hkT)
            nc.scalar.activation(hkT[:], hB[:], AF.Identity,
                                 bias=bi[:], scale=sc[:])
            # heads stage 1: t = hk @ W1.T + b1 (both heads stacked)
            ts = fp.tile([128, NPAD], dt.bfloat16, tag="ts")
            for b, c0, w in for_chunks():
                ps = fpp.tile([128, NC512], dt.float32, tag="ps_t")
                pv = ps[:, :w]
                nc.tensor.matmul(pv, catW1T[64 * b:64 * b + 64, :],
                                 hkT[64 * b:64 * b + 64, c0:c0 + w],
                                 start=True, stop=False)
                bias_mm(pv, catb1, w, False)
                nc.vector.tensor_copy(ts[:, b * BAND + c0:b * BAND + c0 + w], pv)
            if padn:
                nc.vector.memset(ts[:, NPAD - padn:NPAD], 0.0)
            tr = fp.tile([128, NPAD], dt.bfloat16, tag="tr")
            sc2, bi2 = bn_stats_apply(fp, fpp, ts, NPAD, bn_head, False, 2,
                                      scratch=tr)
            nc.scalar.activation(tr[:], ts[:], AF.Relu, bias=bi2[:], scale=sc2[:])
            # heads stage 2: per-chunk psum -> staging -> DMA out slice
            nch2 = [min(NC512, NPAD - i * NC512)
                    for i in range((NPAD + NC512 - 1) // NC512)]
            for ci, w in enumerate(nch2):
                c0 = ci * NC512
                ps = fpp.tile([64, NC512], dt.float32, tag="ps_o")
                nc.tensor.matmul(ps[0:2, :w], aW2T[0:64, :], tr[0:64, c0:c0 + w],
                                 start=True, stop=False)
                nc.tensor.matmul(ps[0:2, :w], ab2[:], ones_f[:, :w],
                                 start=False, stop=True)
                nc.tensor.matmul(ps[32:53, :w], bW2T[64:128, :], tr[64:128, c0:c0 + w],
                                 start=True, stop=False)
                nc.tensor.matmul(ps[32:53, :w], bb2[:], ones_f[:, :w],
                                 start=False, stop=True)
                og = fp.tile([64, NC512], dt.float32, tag="ostage", bufs=2)
                nc.vector.memset(og[:, :w], 0.0)
                nc.vector.tensor_copy(og[0:2, :w], ps[0:2, :w])
                nc.vector.tensor_copy(og[32:53, :w], ps[32:53, :w])
                nc.sync.dma_start(out_d.ap()[:, c0:c0 + w], og[:, :w])

    nc.compile()
    return nc


# ---------------------------------------------------------------- assemble

def assemble(cfg, plan, results):
    C = cfg["NCORES"]; NLOC = cfg["NLOC"]
    xa = np.zeros((cfg["N"], 2), np.float32)
    xb = np.zeros((cfg["N"], 21), np.float32)
    for c in range(C):
        o = results[c]["out"]
        order = plan["orders"][c]
        xa_c = o[0:2, :].T          # [NPAD, 2] in pi-order
        xb_c = o[32:53, :].T
        sl = slice(c * NLOC, (c + 1) * NLOC)
        inv = np.empty(cfg["NPAD"], np.int64)
        inv[order] = np.arange(cfg["NPAD"])
        xa[sl] = xa_c[inv[:NLOC]]
        xb[sl] = xb_c[inv[:NLOC]]
    return xa, xb


def run(cfg, inputs, sim=False, trace=False):
    plan = make_plan(cfg, inputs)
    in_maps = make_in_maps(cfg, plan, inputs)
    nc = build_program(cfg, plan)
    res = None
    if sim:
        from concourse.bass_interp import MultiCoreSim
        msim = MultiCoreSim(nc, num_cores=cfg["NCORES"], trace=False)
        for i in range(cfg["NCORES"]):
            for k, v in in_maps[i].items():
                msim.cores[i].tensor(k)[:] = v
        msim.simulate(check_with_hw=False)
        results = [{"out": np.array(msim.cores[i].tensor("out"))}
                   for i in range(cfg["NCORES"])]
    else:
        from concourse.bass_utils import run_bass_kernel_spmd
        res = run_bass_kernel_spmd(nc, in_maps, list(range(cfg["NCORES"])),
                                   trace=trace)
        results = res.results
    return assemble(cfg, plan, results), res


def kernel(**inputs):
    cfg = make_cfg()
    (xa, xb), _ = run(cfg, inputs, sim=False)
    return xa, xb
